# revision 30
# baseline (speedup 1.0000x reference)
"""Trainium2 Bass kernel for a 3-layer GCN + mean-pool + MLP + softmax.

Reference computation (N=16384 nodes, dense adjacency):
    Ahat = D^-1/2 (A + I) D^-1/2
    H0 = X;  H_{l+1} = relu(Ahat @ (H_l @ W_l) + b_l)   l = 0,1,2
    g = mean(H3, axis=0);  h1 = elu(g @ Wh1 + bh1)
    logits = h1 @ Wh2 + bh2;  probs = softmax(logits)

Distribution (8 NeuronCores, 1D node/row parallel):
  - Host folds the symmetric degree normalization into the adjacency and
    ships each core the *transposed* normalized adjacency columns for its
    2048 output nodes as fp8 e4m3 (32MB/core), pre-tiled to the SBUF
    layout [quarter, rank, partition, stripe, i] so every adjacency DMA
    reads per-partition-contiguous runs.  ASCALE/XSCALE keep fp8 values
    in normal range and are divided back out by the relu's scale.
  - On device, the big matmul per layer streams the adjacency through the
    tensor engine (moving operand, DoubleRow fp8: 256-deep contraction)
    against stationary Y_l = H_l @ W_l tiles:
        out.T[c, i] = sum_j Y_l[j, c] * Ahat.T[j, i]   (PSUM fp32 accum)
  - Layer 1 uses associativity: Ahat @ (X W0) = (Ahat @ X) W0, with X
    itself (fp8, host-tiled) as the stationary — no device-side Y0 and no
    collective before layer 1.
  - SBUF holds 20 of the 32 1MB adjacency groups across all three layers
    (cpool); only 12 groups re-stream per layer through a 2-deep apool.
  - Between layers: QUARTER-granular pipeline.  relu chunk q feeds the
    Y-projection for own-node quarter q immediately, so AllGather #q
    triggers ~1.5us after the last matmul of the layer; the next layer's
    matmul phase q waits only on AG #q, and AGs q>=1 hide behind the
    DMA-paced phases before them.  A tiny mid-layer AllReduce keeps the
    CC stream warm so cross-core skew is absorbed off the critical path.
  - Mean pool: per-relu-chunk partial sums + AllReduce; 1/N is folded
    into Wh1 on the host, elu's "-1" is folded into bh2, and the 2-class
    softmax is computed as sigmoid(+-(l0-l1)).
  - DMA ring split: the bulk adjacency stream runs on the SP (nc.sync)
    HWDGE ring; all small loads that may wait on collectives run on the
    ACT (nc.scalar) ring so they never stall the adjacency stream.
"""

import numpy as np
import ml_dtypes

N = 16384
NCORES = 8
ROWS = N // NCORES          # 2048 output nodes per core
P = 128
DIMS = [64, 32, 48, 64]     # feature dims: in, after l0, l1, l2
NPART = 4                   # node quarters for the pipelined AllGathers
NSTRIPE = 4                 # 128-row j-stripes per DMA group (1MB fp8)
NGROUPS = NCORES * NPART    # 32 groups per layer: (q4, r)
QCH = 512                   # moving-operand free-dim chunk (1 PSUM bank)
NQ = ROWS // QCH            # 4
NU4 = 4                     # 128-node u-tiles per quarter
NDT = NSTRIPE // 2          # 2 double j-tiles per group (DoubleRow)
ASCALE = 16.0               # fp8 range helper for Ahat
XSCALE = 16.0               # fp8 range helper for X
ABUFS = 2                   # streamed adjacency groups in flight (2MB)
NRCACHE = 5                 # ranks cached per quarter (20 groups = 20MB)
# interleaved visit order per quarter: streamed ranks (>=NRCACHE) spread
# between cached ones so the DMA stream never idles while the PE chews
# cached groups, and vice versa
RORDER = [5, 0, 1, 6, 2, 3, 7, 4]
# filler matmuls: keep the PE HAM activity monitor busy through DMA-paced
# stretches so the clock gate stays at 2.4 GHz (idle PE decays to 1.2 GHz
# and cold matmuls pace at ~430ns instead of ~160ns)
FILL_START = 10             # before the first adjacency group lands
FILL_L0 = 6                 # after each layer-0 group (DMA-paced layer)
FILL_TRANS = 18             # across each inter-layer AllGather window

_nc_cache = None


def _build_nc():
    from concourse import bacc, mybir, tile

    dt = mybir.dt
    F32 = dt.float32
    F8 = dt.float8e4
    BF16 = dt.bfloat16
    AF = mybir.ActivationFunctionType
    OP = mybir.AluOpType
    DR = mybir.MatmulPerfMode.DoubleRow

    nc = bacc.Bacc(
        "TRN2", target_bir_lowering=False, debug=False, num_devices=NCORES
    )

    # adjacency pre-tiled on host: [q4, r, p, t, i]
    a_t = nc.dram_tensor(
        "a_t", [NPART, NCORES, P, NSTRIPE, ROWS], F8, kind="ExternalInput"
    )
    # full X (scaled, fp8), pre-tiled partition-major so each partition's
    # stationary data is one contiguous run: [p, q4, r, u, c]
    x8 = nc.dram_tensor(
        "x8", [P, NPART, NCORES, NU4, DIMS[0]], F8, kind="ExternalInput"
    )
    w_d = [
        nc.dram_tensor(
            f"w{l}", [DIMS[l], DIMS[l + 1]], F32 if l == 0 else BF16,
            kind="ExternalInput",
        )
        for l in range(3)
    ]
    b_d = [
        nc.dram_tensor(f"b{l}", [DIMS[l + 1], 1], F32, kind="ExternalInput")
        for l in range(3)
    ]
    wh1_d = nc.dram_tensor("wh1", [DIMS[3], 32], F32, kind="ExternalInput")
    bh1_d = nc.dram_tensor("bh1", [32, 1], F32, kind="ExternalInput")
    # fused head: cols [0:2] logits, cols [64:66] +-logit differences for
    # the sigmoid softmax (offset 64 keeps partition slices 32-aligned)
    w4_d = nc.dram_tensor("w4", [32, 66], F32, kind="ExternalInput")
    b4_d = nc.dram_tensor("b4", [66, 1], F32, kind="ExternalInput")
    logits_o = nc.dram_tensor("logits", [2, 1], F32, kind="ExternalOutput")
    probs_o = nc.dram_tensor("probs", [2, 1], F32, kind="ExternalOutput")

    rg = [list(range(NCORES))]

    with tile.TileContext(nc) as tc:
        with (
            tc.tile_pool(name="const", bufs=1) as const,
            tc.tile_pool(name="apool", bufs=ABUFS) as apool,
            tc.tile_pool(name="cpool", bufs=NPART * NRCACHE) as cpool,
            tc.tile_pool(name="spool", bufs=1) as spool,
            tc.tile_pool(name="hpool", bufs=1) as hpool,
            tc.tile_pool(name="ypool", bufs=2) as ypool,
            tc.tile_pool(name="smal", bufs=1) as smal,
            tc.tile_pool(name="accp", bufs=1, space="PSUM") as accp,
            tc.tile_pool(name="psml", bufs=2, space="PSUM") as psml,
            tc.tile_pool(name="psfill", bufs=1, space="PSUM") as psfill,
            tc.tile_pool(name="psmlp", bufs=1, space="PSUM") as psmlp,
            tc.tile_pool(name="dram", bufs=1, space="DRAM") as dram,
        ):
            # ---- layer-1 stationary = X itself (fp8, host-tiled),
            #      loaded first so the tensor engine can start ASAP ----
            def stat_set(l, c_out):
                return [
                    spool.tile(
                        [P, NCORES, NU4, c_out], F8,
                        tag=f"stat{q4}", name=f"stat{l}_{q4}",
                    )
                    for q4 in range(NPART)
                ]

            stat = stat_set(0, DIMS[0])
            for q4 in range(NPART):
                nc.scalar.dma_start(stat[q4][:], x8.ap()[:, q4])

            # ---- HAM-warming filler machinery (dummy fp8 matmuls into a
            #      dedicated scratch PSUM bank) ----
            dm_w = smal.tile([P, 8], F8, name="dmw")
            nc.vector.memset(dm_w[:], 0.0)
            dm_x = smal.tile([P, QCH], F8, name="dmx")
            nc.vector.memset(dm_x[:], 0.0)
            psf = psfill.tile([8, QCH], F32, tag="fill", name="psf")

            def fillers(n):
                for _ in range(n):
                    nc.tensor.matmul(
                        psf[:], lhsT=dm_w[:], rhs=dm_x[:],
                        start=True, stop=True,
                    )

            fillers(FILL_START)

            # ---- constants into SBUF (ACT ring — keep SP ring for A) ----
            def load(handle, shape, name, dtype=F32):
                t = const.tile(shape, dtype, name=name)
                nc.scalar.dma_start(t[:], handle.ap())
                return t

            w_sb = [
                load(
                    w_d[l], [DIMS[l], DIMS[l + 1]], f"w{l}sb",
                    dtype=F32 if l == 0 else BF16,
                )
                for l in range(3)
            ]
            b_sb = [load(b_d[l], [DIMS[l + 1], 1], f"b{l}sb") for l in range(3)]
            wh1_sb = load(wh1_d, [DIMS[3], 32], "wh1sb")
            bh1_sb = load(bh1_d, [32, 1], "bh1sb")
            w4_sb = load(w4_d, [32, 66], "w4sb")
            b4_sb = load(b4_d, [66, 1], "b4sb")

            def resync(tag, dep_ap):
                # chain the trigger on `dep_ap` (a mid-layer adjacency tile)
                # so every rank fires this at the same point in its layer
                rs_src = smal.tile([1, 1], F32, name=f"rss_{tag}")
                nc.vector.tensor_copy(out=rs_src[:], in_=dep_ap)
                rs_in = dram.tile([1, 1], F32, name=f"rsin_{tag}")
                nc.scalar.dma_start(rs_in[:], rs_src[:])
                rs_out = dram.tile([1, 1], F32, name=f"rsout_{tag}")
                nc.gpsimd.collective_compute(
                    "AllReduce",
                    OP.add,
                    replica_groups=rg,
                    ins=[rs_in[:].opt()],
                    outs=[rs_out[:].opt()],
                )

            h_sb = None
            gp_q = None
            a_cached = {}
            for l in range(3):
                c_stat = DIMS[0] if l == 0 else DIMS[l + 1]
                c_out = DIMS[l + 1]
                acc = [
                    accp.tile([P, QCH], F32, tag=f"acc{q}", name=f"acc{l}_{q}")
                    for q in range(NQ)
                ]

                def acc_sl(q, c):
                    return acc[q][:c, :]

                gi = 0
                for q4 in range(NPART):
                    for r in RORDER:
                        # layer 0 is DMA-paced: alternate its adjacency
                        # loads across both HWDGE rings (SP + ACT) so the
                        # ~1us per-DMA completion gaps of the two FIFOs
                        # overlap.  Layers 1-2 keep everything on SP so the
                        # ACT ring never stalls the stream behind a
                        # collective-gated stationary load.
                        eng = nc.scalar if (l == 0 and gi % 2 == 1) else nc.sync
                        if r < NRCACHE:
                            if l == 0:
                                a_sb = cpool.tile(
                                    [P, NSTRIPE, ROWS], F8, tag="ac",
                                    name=f"ac{q4}_{r}",
                                )
                                eng.dma_start(a_sb[:], a_t.ap()[q4, r])
                                a_cached[(q4, r)] = a_sb
                            else:
                                a_sb = a_cached[(q4, r)]
                        else:
                            a_sb = apool.tile(
                                [P, NSTRIPE, ROWS], F8, tag="a",
                                name=f"a{l}_{q4}_{r}",
                            )
                            eng.dma_start(a_sb[:], a_t.ap()[q4, r])
                        for t2 in range(NDT):
                            lw = stat[q4][:, r, 2 * t2 : 2 * t2 + 2, :]
                            for q in range(NQ):
                                nc.tensor.matmul(
                                    acc_sl(q, c_stat),
                                    lhsT=lw,
                                    rhs=a_sb[
                                        :, 2 * t2 : 2 * t2 + 2,
                                        q * QCH : (q + 1) * QCH,
                                    ],
                                    start=(gi == 0 and t2 == 0),
                                    stop=(gi == NGROUPS - 1 and t2 == NDT - 1),
                                    perf_mode=DR,
                                )
                        if l == 0:
                            fillers(FILL_L0)
                        if gi == 16:
                            resync(f"rs{l}", a_sb[0:1, 0, 0:1])
                        gi += 1

                # ---- layer epilogue: relu chunk q -> (Yproj + AllGather of
                #      own-node quarter q) | (partial mean-pool on l==2) ----
                h_sb = hpool.tile([c_out, ROWS], BF16, tag="h", name=f"h{l}")
                if l < 2:
                    c_next = DIMS[l + 2]
                    stat = stat_set(l + 1, c_next)
                if l == 2:
                    gp_q = smal.tile([DIMS[3], NQ], F32, name="gpart")
                for q in range(NQ):
                    if l == 0:
                        # H1 chunk = relu((Ahat@X)chunk @ W0 / s + b0)
                        p1 = ypool.tile(
                            [DIMS[0], QCH], F32, tag="p1", name=f"p1_{q}"
                        )
                        nc.vector.tensor_copy(out=p1[:], in_=acc_sl(q, DIMS[0]))
                        ps2 = psml.tile(
                            [DIMS[1], QCH], F32, tag="psy", name=f"ps2_{q}"
                        )
                        nc.tensor.matmul(
                            ps2[:], lhsT=w_sb[0][:], rhs=p1[:],
                            start=True, stop=True,
                        )
                        nc.scalar.activation(
                            h_sb[:, q * QCH : (q + 1) * QCH],
                            ps2[:],
                            AF.Relu,
                            bias=b_sb[0][:],
                            scale=1.0 / (ASCALE * XSCALE),
                        )
                    else:
                        # on the last layer the relu also emits its own
                        # free-axis sum (accum_out) = this chunk's mean-pool
                        # partial, so no separate reduce sits on the tail
                        nc.scalar.activation(
                            h_sb[:, q * QCH : (q + 1) * QCH],
                            acc_sl(q, c_out),
                            AF.Relu,
                            bias=b_sb[l][:],
                            scale=1.0 / ASCALE,
                            accum_out=(
                                gp_q[:, q : q + 1] if l == 2 else None
                            ),
                        )
                    if l == 2:
                        continue
                    # own-node quarter q: Y-projection + AllGather
                    y_sb = ypool.tile(
                        [P, NU4, c_next], F8, tag="y", name=f"y{l}_{q}"
                    )
                    for u in range(NU4):
                        ug = q * NU4 + u
                        ps = psml.tile(
                            [P, c_next], F32, tag="psy", name=f"psy{l}_{ug}"
                        )
                        nc.tensor.matmul(
                            ps[:],
                            lhsT=h_sb[:, ug * P : (ug + 1) * P],
                            rhs=w_sb[l + 1][:],
                            start=True,
                            stop=True,
                        )
                        nc.vector.tensor_copy(out=y_sb[:, u, :], in_=ps[:])
                    ag_in = dram.tile([P, NU4, c_next], F8, name=f"agin{l}_{q}")
                    ag_out = dram.tile(
                        [NCORES, P, NU4, c_next], F8, name=f"agout{l}_{q}",
                        addr_space="Shared",
                    )
                    nc.scalar.dma_start(ag_in[:], y_sb[:])
                    nc.gpsimd.collective_compute(
                        "AllGather",
                        OP.bypass,
                        replica_groups=rg,
                        ins=[ag_in[:].opt()],
                        outs=[ag_out[:].opt()],
                    )
                    nc.scalar.dma_start(
                        stat[q][:], ag_out[:].rearrange("r p u c -> p r u c")
                    )
                if l < 2:
                    # keep the PE clock warm across the AllGather window
                    fillers(FILL_TRANS)

            # ---- mean pool over all nodes (1/N folded into Wh1) ----
            gp = smal.tile([DIMS[3], 1], F32, name="gsum")
            nc.vector.tensor_reduce(
                gp[:], gp_q[:], axis=mybir.AxisListType.X, op=OP.add
            )
            ar_in = dram.tile([DIMS[3], 1], F32, name="arin")
            ar_out = dram.tile([DIMS[3], 1], F32, name="arout", addr_space="Shared")
            nc.scalar.dma_start(ar_in[:], gp[:])
            nc.gpsimd.collective_compute(
                "AllReduce",
                OP.add,
                replica_groups=rg,
                ins=[ar_in[:].opt()],
                outs=[ar_out[:].opt()],
            )
            g_sb = smal.tile([DIMS[3], 1], F32, name="gsb")
            nc.scalar.dma_start(g_sb[:], ar_out[:])

            # ---- MLP head: h1 = elu(g @ Wh1 + bh1); the -1 of elu is
            #      folded into bh2 on the host ----
            ps1 = psmlp.tile([32, 1], F32, tag="mlp", name="ps1")
            nc.tensor.matmul(ps1[:], lhsT=wh1_sb[:], rhs=g_sb[:], start=True, stop=True)
            # elu(x) + 1 = relu(x) + exp(min(x, 0))
            tmin = smal.tile([32, 1], F32, name="tmin")
            nc.vector.tensor_scalar(tmin[:], ps1[:], bh1_sb[:], 0.0, OP.add, OP.min)
            e1 = smal.tile([32, 1], F32, name="e1")
            nc.scalar.activation(e1[:], tmin[:], AF.Exp)
            r1 = smal.tile([32, 1], F32, name="r1")
            nc.scalar.activation(r1[:], ps1[:], AF.Relu, bias=bh1_sb[:])
            h1 = smal.tile([32, 1], F32, name="h1")
            nc.vector.tensor_tensor(h1[:], e1[:], r1[:], OP.add)

            # ---- fused head: one matmul gives logits [0:2] and +-logit
            #      differences [64:66]; probs = sigmoid(differences) ----
            ps2m = psmlp.tile([66, 1], F32, tag="mlp", name="ps2m")
            nc.tensor.matmul(ps2m[:], lhsT=w4_sb[:], rhs=h1[:], start=True, stop=True)
            out66 = smal.tile([66, 1], F32, name="out66")
            nc.vector.tensor_scalar(out66[:], ps2m[:], b4_sb[:], None, OP.add)
            nc.scalar.dma_start(logits_o.ap(), out66[0:2, :])
            probs_sb = smal.tile([2, 1], F32, name="probssb")
            nc.scalar.activation(probs_sb[:], out66[64:66, :], AF.Sigmoid)
            nc.scalar.dma_start(probs_o.ap(), probs_sb[:])

    nc.finalize()
    return nc


def _install_ntff_hook():
    """Register the axon NTFF profiling hook if the container's antenv stub
    lacks it (bass_utils imports antenv.axon_hooks when trace=True)."""
    import sys
    import types

    try:
        import antenv.axon_hooks  # noqa: F401
        return
    except ImportError:
        pass
    mod = types.ModuleType("antenv.axon_hooks")
    _h = [None]
    mod.set_axon_ntff_profile_hook = lambda h: _h.__setitem__(0, h)
    mod.get_axon_ntff_profile_hook = lambda: _h[0]
    sys.modules["antenv.axon_hooks"] = mod
    import antenv

    antenv.axon_hooks = mod
    try:
        from trn_agent_boot import trn_boot

        hook = trn_boot._ntff_profile_via_ctypes("/opt/axon/libaxon_pjrt.so")
        if hook is not None:
            mod.set_axon_ntff_profile_hook(hook)
    except Exception:
        pass


def _get_nc():
    global _nc_cache
    if _nc_cache is None:
        _nc_cache = _build_nc()
    return _nc_cache


_last_results = None


def kernel(
    node_feat,
    adj_matrix,
    W0,
    b0,
    W1,
    b1,
    W2,
    b2,
    Wh1,
    bh1,
    Wh2,
    bh2,
):
    global _last_results
    import os

    node_feat = np.ascontiguousarray(np.asarray(node_feat, dtype=np.float32))
    adj = np.asarray(adj_matrix, dtype=np.float32)

    # ---- host-side sharding / preprocessing ----
    deg = adj.sum(axis=1, dtype=np.float32) + 1.0
    dinv = (1.0 / np.sqrt(deg)).astype(np.float32)

    fp8 = ml_dtypes.float8_e4m3
    bf16 = ml_dtypes.bfloat16
    f32c = lambda a, shape=None: np.ascontiguousarray(
        np.asarray(a, dtype=np.float32).reshape(shape)
        if shape is not None
        else np.asarray(a, dtype=np.float32)
    )

    # X scaled to fp8, tiled [p, q4, r, u, c]: node j = r*2048+q4*512+u*128+p
    x8 = (node_feat * np.float32(XSCALE)).astype(fp8)
    x8 = np.ascontiguousarray(
        x8.reshape(NCORES, NPART, NU4, P, DIMS[0]).transpose(3, 1, 0, 2, 4)
    )

    wh1 = np.asarray(Wh1, np.float32) / np.float32(N)   # fold the mean's 1/N
    wh2 = np.asarray(Wh2, np.float32)
    # elu's "-1" folded into the head bias; sigmoid softmax via +-differences
    bh2f = np.asarray(bh2, np.float32) - wh2.sum(axis=0)
    m22 = np.array([[1.0, -1.0], [-1.0, 1.0]], dtype=np.float32)
    w4 = np.zeros((32, 66), dtype=np.float32)
    w4[:, 0:2] = wh2
    w4[:, 64:66] = wh2 @ m22
    b4 = np.zeros((66, 1), dtype=np.float32)
    b4[0:2, 0] = bh2f
    b4[64:66, 0] = m22 @ bh2f

    common = {
        "x8": x8,
        "w0": f32c(W0),
        "b0": f32c(b0, (-1, 1)),
        "w1": np.ascontiguousarray(np.asarray(W1, np.float32)).astype(bf16),
        "b1": f32c(b1, (-1, 1)),
        "w2": np.ascontiguousarray(np.asarray(W2, np.float32)).astype(bf16),
        "b2": f32c(b2, (-1, 1)),
        "wh1": f32c(wh1),
        "bh1": f32c(bh1, (-1, 1)),
        "w4": w4,
        "b4": b4,
    }

    in_maps = []
    idx = np.arange(ROWS)
    sdinv = dinv * np.float32(ASCALE)
    for k in range(NCORES):
        sl = slice(k * ROWS, (k + 1) * ROWS)
        # rows of ASCALE*Ahat for this core's output nodes
        blk = adj[sl, :] * sdinv[sl, None]
        blk *= dinv[None, :]
        blk[idx, k * ROWS + idx] = sdinv[sl] * dinv[sl]  # + I self loops
        a_k = blk.T.astype(fp8)  # [N, ROWS] = scaled Ahat.T cols
        # pre-tile to device layout [q4, r, p, t, i]:
        # row j = r*2048 + q4*512 + t*128 + p
        a_k = np.ascontiguousarray(
            a_k.reshape(NCORES, NPART, NSTRIPE, P, ROWS).transpose(1, 0, 3, 2, 4)
        )
        m = {"a_t": a_k}
        m.update(common)
        in_maps.append(m)

    from concourse import bass_utils

    nc = _get_nc()
    trace = bool(int(os.environ.get("GCN_TRACE", "0")))
    if trace:
        _install_ntff_hook()
    res = bass_utils.run_bass_kernel_spmd(
        nc, in_maps, core_ids=list(range(NCORES)), trace=trace
    )
    _last_results = res

    out0 = res.results[0]
    logits = np.asarray(out0["logits"], dtype=np.float32).reshape(2)
    probs = np.asarray(out0["probs"], dtype=np.float32).reshape(2)
    return (logits, probs)
